# revision 43
# baseline (speedup 1.0000x reference)
"""Trainium2 Bass kernel: causal multi-head attention block (B=2,S=2048,H=2048,NH=16,HD=128).

Sharding: 8 cores = DP over batch (2) x TP over head-groups (4 groups of 4 heads).
Each core computes q/k/v projections for its 4 heads, RoPE, causal softmax
attention, and a partial output projection; the host sums the 4 partials per
batch and adds bo.

Device layouts (all chosen so every matmul streams natural contiguous-free
tiles; the host pre-transposes x and the weights and casts matmul operands to
bf16 -- accumulation stays fp32 in PSUM):
  xT   [H=2048(f), S=2048(s)]   = x[b].T                       bf16
  wqT  [2048(f), 512(d)]        = (Wq[rows]/sqrt(HD)).T        bf16
  wkT  [2048(f), 512(d)]        = Wk[rows].T                   bf16
  wvT  [2048(f), 512(d)]        = Wv[rows].T                   bf16
  woT  [512(d), 2048(o)]        = Wo[:, rows].T                bf16
Attention runs with transposed score tiles ST[k,q] so the P@V matmul needs no
on-chip transposes; the softmax denominators come from an all-ones [128,128]
stationary matmul, which lands the row sums broadcast across every PSUM
partition so 1/denom is a single DVE reciprocal.

Schedule notes:
 - x is DMA'd once into a persistent 8 MiB SBUF cache ([128, ft, s] per
   512-wide s-block) and reused by both the Q/K and the V projections.
 - each dma_start occupies its issuing sequencer for ~2.2us + transfer time,
   so DMAs are split across two queues (SP/HWDGE for weights+y, Pool/SWDGE
   for x+y) and kept few and large, chunked only where startup pacing needs.
 - RoPE's rotate_half runs on the DVE as a partition pair-swap stream_shuffle
   with the sign folded into the host-built sin table (no PE matmul).
 - Q/K projection accumulators are processed head-PAIR-major so PSUM banks
   drain (bias+RoPE) while the next pair's matmuls stream.
 - the attention inner loop keeps a 4-deep score-matmul pipeline (2 PSUM
   banks from the score pool + 2 borrowed from the idle out-proj pool), so
   the PE never waits on the mask-add + Exp chain.
 - output-projection PSUM is drained by the Scalar engine (DVE is busier)
   and y is stored/DMA'd as bf16; the host accumulates partials in f32.
"""

import math
import os
import sys

import numpy as np

for _p in ("/opt/trn_rl_repo",):
    if _p not in sys.path and os.path.isdir(_p):
        sys.path.insert(0, _p)

import ml_dtypes

import concourse.bass as bass
import concourse.mybir as mybir
import concourse.tile as tile
from concourse import bacc

B, S, H, NH, HD = 2, 2048, 2048, 16, 128
NCORES = 8
HG = 4            # head-groups (TP degree)
HPG = NH // HG    # heads per group = 4
DLOC = HPG * HD   # local d width = 512
FT = H // 128     # 16 f-tiles
SJ = S // 512     # 4 s/q tiles of 512
KT128 = S // 128  # 16 k-tiles of 128
NEG = -1e30
WCH = 4           # f-tiles per startup DMA chunk (512 KiB)

F32 = mybir.dt.float32
F16 = mybir.dt.float16
BF16 = mybir.dt.bfloat16
NPBF16 = ml_dtypes.bfloat16
YDT = BF16        # on-device y dtype (host accumulates in f32)
NPYDT = NPBF16


def build_program(mode: str) -> bass.Bass:
    """mode in {'causal', 'full', 'bias'}"""
    # Collapse Tile's DMA-completion sem round-robin to one lane per DGE class.
    # SP-issued HWDGE DMAs retire FIFO, so a single counting sem stays sound,
    # and every consumer's DMA waits merge into one sem -- without this, waits
    # on 3+ distinct DMAHW*/DMASW* sems overflow the ISA sync-wait budget on
    # DMA instructions ("Too many sync wait commands" in walrus codegen).
    import concourse.tile_sem_assignment as tsa

    tsa.NUM_HWDGE_SEMS = 1
    tsa.NUM_SWDGE_GLOBAL_SEMS = 1
    nc = bacc.Bacc()
    xT = nc.dram_tensor("xT", [H, S], BF16, kind="ExternalInput")
    wqT = nc.dram_tensor("wqT", [H, DLOC], BF16, kind="ExternalInput")
    wkT = nc.dram_tensor("wkT", [H, DLOC], BF16, kind="ExternalInput")
    wvT = nc.dram_tensor("wvT", [H, DLOC], BF16, kind="ExternalInput")
    woT = nc.dram_tensor("woT", [DLOC, H], BF16, kind="ExternalInput")
    bqT = nc.dram_tensor("bqT", [128, HPG], F32, kind="ExternalInput")
    bkT = nc.dram_tensor("bkT", [128, HPG], F32, kind="ExternalInput")
    bv = nc.dram_tensor("bv", [128, DLOC], F32, kind="ExternalInput")
    cosT = nc.dram_tensor("cosT", [HD, S], BF16, kind="ExternalInput")
    # sinT is sign-folded on the host: row 2i holds -sin, row 2i+1 holds +sin,
    # so RoPE's rotate_half is a partition pair-swap (DVE stream_shuffle).
    sinT = nc.dram_tensor("sinT", [HD, S], BF16, kind="ExternalInput")
    if mode == "causal":
        dbias = nc.dram_tensor("dbias", [4, 128, 512], F32, kind="ExternalInput")
    elif mode == "bias":
        fbias = nc.dram_tensor("fbias", [S, S], F32, kind="ExternalInput")
    y = nc.dram_tensor("y", [S, H], YDT, kind="ExternalOutput")

    with tile.TileContext(nc) as tc:
        with (
            tc.tile_pool(name="qt", bufs=HPG * SJ) as qt_pool,
            tc.tile_pool(name="kt", bufs=HPG * SJ) as kt_pool,
            tc.tile_pool(name="vt", bufs=KT128) as vt_pool,
            tc.tile_pool(name="xin", bufs=SJ) as xin_pool,
            tc.tile_pool(name="consts", bufs=1) as consts,
        ):
            QT = {}  # (h, sj) -> [128, 512] bf16 tile, RoPE'd q^T (pre-scaled)
            KT = {}  # (h, sj) -> [128, 512] bf16 tile, RoPE'd k^T
            VT = {}  # ssub -> [128(s), 512(d)] bf16 tile, v + bias

            ones_sb = consts.tile([128, 128], BF16, tag="ones")
            nc.vector.memset(ones_sb[:], 1.0)
            # warmup: pull the ACT function-table load into the startup DMA
            # shadow (Exp's set also contains Identity/Copy)
            warm_sb = consts.tile([1, 1], F32, tag="warm")
            nc.scalar.activation(
                warm_sb[:], ones_sb[0:1, 0:1], mybir.ActivationFunctionType.Exp
            )
            bv_sb = consts.tile([128, DLOC], F32, tag="bv")
            wo_sb = consts.tile([128, HPG, H], BF16, tag="wo")
            db_sb = None
            if mode == "causal":
                db_sb = consts.tile([128, 4, 512], F32, tag="db")

            # persistent x cache: [128, ft, s-block] per 512-wide s-block
            xts = [
                xin_pool.tile([128, FT, 512], BF16, tag="xt", name="xt")
                for _ in range(SJ)
            ]

            # ================= Phase 1: Q/K projections + RoPE =================
            with (
                tc.tile_pool(name="ps12", bufs=8, space="PSUM") as psum,
                tc.tile_pool(name="wqk", bufs=1) as wqk_pool,
                tc.tile_pool(name="csn", bufs=1) as csn_pool,
                tc.tile_pool(name="rtmp", bufs=4) as rtmp_pool,
            ):
                wq_sb = wqk_pool.tile([128, FT, DLOC], BF16, tag="wq")
                wk_sb = wqk_pool.tile([128, FT, DLOC], BF16, tag="wk")
                cos_sb = csn_pool.tile([HD, S], BF16, tag="cos")
                sin_sb = csn_pool.tile([HD, S], BF16, tag="sin")
                bq_sb = consts.tile([128, HPG], F32, tag="bq")
                bk_sb = consts.tile([128, HPG], F32, tag="bk")

                # Two parallel DMA issue streams (each dma_start costs its
                # issuing sequencer ~2us fixed + the transfer time, serially):
                #   SP/HWDGE:   weights + small consts
                #   Pool/SWDGE: the x tiles
                def wchunk(eng, dst, src, f0, n):
                    eng.dma_start(
                        dst[:, f0 : f0 + n, :],
                        src[f0 * 128 : (f0 + n) * 128, :].rearrange(
                            "(ft p) d -> p ft d", p=128
                        ),
                    )

                def xchunk(sj, f0, n):
                    nc.gpsimd.dma_start(
                        xts[sj][:, f0 : f0 + n, :],
                        xT[
                            f0 * 128 : (f0 + n) * 128,
                            sj * 512 : (sj + 1) * 512,
                        ].rearrange("(ft p) s -> p ft s", p=128),
                    )

                # Pool queue: x first (paces the first q matmuls), then the
                # first half of wk (k consumption starts at ~19us)
                xchunk(0, 0, 4)
                xchunk(0, 4, 4)
                xchunk(0, 8, FT // 2)
                wchunk(nc.gpsimd, wk_sb, wkT, 0, FT // 2)
                for sj in range(1, SJ):
                    xchunk(sj, 0, FT)
                # SP queue: wq pieces, second wk half, small consts
                wchunk(nc.sync, wq_sb, wqT, 0, 4)
                wchunk(nc.sync, wq_sb, wqT, 4, 4)
                wchunk(nc.sync, wq_sb, wqT, 8, FT // 2)
                wchunk(nc.sync, wk_sb, wkT, FT // 2, FT // 2)
                nc.sync.dma_start(bq_sb[:], bqT[:])
                nc.sync.dma_start(bk_sb[:], bkT[:])
                nc.sync.dma_start(cos_sb[:], cosT[:])
                nc.sync.dma_start(sin_sb[:], sinT[:])

                # head-PAIR-major: a pair's two PSUM accumulators complete
                # every 32 matmuls, so bias/RoPE for one pair pipeline under
                # the next pair's matmuls (full ft-major would serialize all
                # 8 ACT identities at the block end; full h-major starves the
                # startup DMA pacing).
                # RoPE rotate_half = swap adjacent partition pairs (the sign
                # lives in the host-folded sin table)
                swap_mask = [p ^ 1 for p in range(32)]

                def rope(ps, h, bias_sb, store, sj, pool, css, sss):
                    t = pool.tile([128, 512], BF16, tag="t", name="qkt")
                    # t = bf16(psum + per-partition bias); frees the bank
                    nc.scalar.activation(
                        t[:],
                        ps[:],
                        mybir.ActivationFunctionType.Identity,
                        bias=bias_sb[:, h : h + 1],
                    )
                    sh = rtmp_pool.tile([128, 512], BF16, tag="sh", name="sh")
                    nc.vector.stream_shuffle(sh[:], t[:], swap_mask)
                    tmp = rtmp_pool.tile([128, 512], BF16, tag="tmp", name="tmp")
                    nc.vector.tensor_mul(tmp[:], sh[:], sss)
                    nc.vector.tensor_mul(t[:], t[:], css)
                    nc.vector.tensor_add(t[:], t[:], tmp[:])
                    store[(h, sj)] = t

                for sj in range(SJ):
                    xt = xts[sj]
                    css = cos_sb[:, sj * 512 : (sj + 1) * 512]
                    sss = sin_sb[:, sj * 512 : (sj + 1) * 512]
                    for which, w_sb, bias_sb, store, pool in (
                        ("q", wq_sb, bq_sb, QT, qt_pool),
                        ("k", wk_sb, bk_sb, KT, kt_pool),
                    ):
                        for hp in range(HPG // 2):
                            pair = (2 * hp, 2 * hp + 1)
                            pp = {
                                h: psum.tile([128, 512], F32, tag="ps", name="ps")
                                for h in pair
                            }
                            for ft in range(FT):
                                for h in pair:
                                    nc.tensor.matmul(
                                        pp[h][:],
                                        w_sb[:, ft, h * 128 : (h + 1) * 128],
                                        xt[:, ft, :],
                                        start=(ft == 0),
                                        stop=(ft == FT - 1),
                                    )
                            for h in pair:
                                rope(pp[h], h, bias_sb, store, sj, pool, css, sss)

                # ================= Phase 2: V projection =================
                # reuses the resident x cache -- no x reload.
                with tc.tile_pool(name="wv", bufs=1) as wv_pool:
                    wv_sb = wv_pool.tile([128, FT, DLOC], BF16, tag="wv")
                    nc.sync.dma_start(
                        wv_sb[:], wvT.rearrange("(ft p) d -> p ft d", p=128)
                    )
                    nc.sync.dma_start(bv_sb[:], bv[:])
                    # phase-3 constants ride behind phase-2's weights on the FIFO
                    if mode == "causal":
                        nc.sync.dma_start(
                            db_sb[:], dbias.rearrange("a p t -> p a t")
                        )
                    nc.sync.dma_start(
                        wo_sb[:], woT.rearrange("(dt p) o -> p dt o", p=128)
                    )
                    for ss in range(KT128):
                        sj, cc = divmod(ss, 4)
                        vp = psum.tile([128, 512], F32, tag="ps", name="ps")
                        for ft in range(FT):
                            nc.tensor.matmul(
                                vp[:],
                                xts[sj][:, ft, cc * 128 : (cc + 1) * 128],
                                wv_sb[:, ft, :],
                                start=(ft == 0),
                                stop=(ft == FT - 1),
                            )
                        v = vt_pool.tile([128, DLOC], BF16, tag="v", name="v")
                        nc.vector.tensor_add(v[:], vp[:], bv_sb[:])
                        VT[ss] = v

            # ============ Phase 3: attention + output projection ============
            with (
                tc.tile_pool(name="pst", bufs=2, space="PSUM") as psum_st,
                tc.tile_pool(name="ppv", bufs=2, space="PSUM") as psum_pv,
                tc.tile_pool(name="pdn", bufs=2, space="PSUM") as psum_dn,
                tc.tile_pool(name="pyp", bufs=2, space="PSUM") as psum_yp,
                tc.tile_pool(name="ex", bufs=8) as exp_pool,
                tc.tile_pool(name="ot", bufs=2 * HPG) as ot_pool,
                tc.tile_pool(name="rc", bufs=4) as rc_pool,
                tc.tile_pool(name="ysb", bufs=4) as y_pool,
                tc.tile_pool(name="fb", bufs=3) as fb_pool,
            ):
                for qj in range(SJ):
                    kmax = 4 * qj + 4 if mode == "causal" else KT128
                    seq = [(h, kj) for h in range(HPG) for kj in range(kmax)]
                    ST = {}
                    PV = {}
                    DN = {}
                    OT = {}

                    def issue_st(i):
                        h, kj = seq[i]
                        # columns of this q-tile that the k-tile can see at
                        # all (causal): the diagonal k-tile only reaches
                        # q >= its own first row.
                        a = kj - 4 * qj
                        off = 128 * a if (mode == "causal" and a > 0) else 0
                        # alternate pools (pyp is idle during attention) so a
                        # 2-deep score pipeline fits the 8 PSUM banks; the
                        # borrowed slots share pyp's "yp" tag/ring
                        if i % 2 == 0:
                            st = psum_st.tile([128, 512], F32, tag="st", name="st")
                        else:
                            st = psum_yp.tile([128, 512], F32, tag="yp", name="st")
                        nc.tensor.matmul(
                            st[:, off:],
                            KT[(h, kj // 4)][:, (kj % 4) * 128 : (kj % 4 + 1) * 128],
                            QT[(h, qj)][:, off:],
                            start=True,
                            stop=True,
                        )
                        ST[i] = (st, off)

                    def normalize(h):
                        # DN[h] holds the denominators broadcast across all
                        # 128 partitions (all-ones stationary), so 1/dn is a
                        # single DVE reciprocal and ot a single multiply.
                        rcb = rc_pool.tile([128, 512], F32, tag="rcb", name="rcb")
                        nc.vector.reciprocal_approx_fast(rcb[:], DN[h][:])
                        ot = ot_pool.tile([128, 512], BF16, tag="ot", name="ot")
                        nc.vector.tensor_mul(ot[:], PV[h][:], rcb[:])
                        OT[h] = ot

                    DEPTH = 4  # st ring slots: 2 in pst + 2 borrowed from pyp
                    for i in range(min(DEPTH, len(seq))):
                        issue_st(i)
                    for i, (h, kj) in enumerate(seq):
                        st, off = ST.pop(i)
                        a = kj - 4 * qj
                        if mode == "causal" and a >= 0:
                            # only the 128-wide band straddling the diagonal
                            # is partially masked
                            nc.vector.tensor_add(
                                st[:, off : off + 128],
                                st[:, off : off + 128],
                                db_sb[:, a, off : off + 128],
                            )
                        elif mode == "bias":
                            fb = fb_pool.tile([128, 512], F32, tag="fb", name="fb")
                            nc.sync.dma_start(
                                fb[:],
                                fbias[
                                    kj * 128 : (kj + 1) * 128,
                                    qj * 512 : (qj + 1) * 512,
                                ],
                            )
                            nc.vector.tensor_add(st[:], st[:], fb[:])
                        e = exp_pool.tile([128, 512], BF16, tag="e", name="e")
                        nc.scalar.activation(
                            e[:, off:], st[:, off:],
                            mybir.ActivationFunctionType.Exp,
                        )
                        if kj == 0:
                            PV[h] = psum_pv.tile([128, 512], F32, tag="pv", name="pv")
                            DN[h] = psum_dn.tile([128, 512], F32, tag="dn", name="dn")
                        nc.tensor.matmul(
                            PV[h][:, off:],
                            VT[kj][:, h * 128 : (h + 1) * 128],
                            e[:, off:],
                            start=(kj == 0),
                            stop=(kj == kmax - 1),
                        )
                        nc.tensor.matmul(
                            DN[h][:, off:],
                            ones_sb[:],
                            e[:, off:],
                            start=(kj == 0),
                            stop=(kj == kmax - 1),
                        )
                        if i + DEPTH < len(seq):
                            issue_st(i + DEPTH)
                        if kj == kmax - 1:
                            normalize(h)
                    # output projection for this q-tile of 512 rows; each
                    # 128-row block drains via one 512 KiB DMA
                    for ss in range(4):
                        last_block = qj == SJ - 1 and ss == 3
                        ysb = y_pool.tile([128, H], YDT, tag="y", name="y")
                        for oj in range(4):
                            yp = psum_yp.tile([128, 512], F32, tag="yp", name="yp")
                            for dt in range(HPG):
                                nc.tensor.matmul(
                                    yp[:],
                                    OT[dt][:, ss * 128 : (ss + 1) * 128],
                                    wo_sb[:, dt, oj * 512 : (oj + 1) * 512],
                                    start=(dt == 0),
                                    stop=(dt == HPG - 1),
                                )
                            if last_block and oj % 2 == 1:
                                # tail: alternate the copies onto DVE so the
                                # final chain isn't serialized on ACT
                                nc.vector.tensor_copy(
                                    ysb[:, oj * 512 : (oj + 1) * 512], yp[:]
                                )
                            else:
                                nc.scalar.activation(
                                    ysb[:, oj * 512 : (oj + 1) * 512],
                                    yp[:],
                                    mybir.ActivationFunctionType.Copy,
                                )
                        r0 = qj * 512 + ss * 128
                        # alternate issue queues: a y DMA costs its sequencer
                        # ~3.7us serially, which would gate the out-proj on SP
                        if last_block:
                            # split the final DMA across both queues so the
                            # two issue latencies overlap
                            nc.sync.dma_start(
                                y[r0 : r0 + 128, : H // 2], ysb[:, : H // 2]
                            )
                            nc.gpsimd.dma_start(
                                y[r0 : r0 + 128, H // 2 :], ysb[:, H // 2 :]
                            )
                        else:
                            eng = nc.sync if (qj * 4 + ss) % 2 == 0 else nc.gpsimd
                            eng.dma_start(y[r0 : r0 + 128, :], ysb[:])
    nc.compile()
    return nc


_PROGRAM_CACHE = {}


def _get_program(mode):
    if mode not in _PROGRAM_CACHE:
        _PROGRAM_CACHE[mode] = build_program(mode)
    return _PROGRAM_CACHE[mode]


def _detect_mode(attn_mask):
    m = np.asarray(attn_mask).reshape(S, S)
    if (m == np.tril(np.ones((S, S), m.dtype))).all():
        return "causal"
    if (m != 0).all():
        return "full"
    return "bias"


def _diag_bias():
    a = np.arange(4)[:, None, None]
    p = np.arange(128)[None, :, None]
    t = np.arange(512)[None, None, :]
    return np.where(128 * a + p <= t, 0.0, NEG).astype(np.float32)


def _bf16(a):
    return np.ascontiguousarray(a).astype(NPBF16)


def build_in_maps(inputs, mode):
    x = np.asarray(inputs["x"], np.float32)
    fcos = np.asarray(inputs["fcos"], np.float32)
    fsin = np.asarray(inputs["fsin"], np.float32)
    Wq, bq = np.asarray(inputs["Wq"], np.float32), np.asarray(inputs["bq"], np.float32)
    Wk, bk = np.asarray(inputs["Wk"], np.float32), np.asarray(inputs["bk"], np.float32)
    Wv, bv = np.asarray(inputs["Wv"], np.float32), np.asarray(inputs["bv"], np.float32)
    Wo = np.asarray(inputs["Wo"], np.float32)
    attn_mask = inputs["attn_mask"]

    sc = 1.0 / math.sqrt(HD)
    sgn = np.where(np.arange(HD) % 2 == 0, -1.0, 1.0).astype(np.float32)[:, None]
    shared = {
        "cosT": _bf16(fcos.T),
        "sinT": _bf16(fsin.T * sgn),
    }
    if mode == "causal":
        shared["dbias"] = _diag_bias()
    elif mode == "bias":
        m = np.asarray(attn_mask).reshape(S, S)
        shared["fbias"] = np.ascontiguousarray(
            np.where(m.T == 0, NEG, 0.0).astype(np.float32)
        )

    in_maps = []
    for c in range(NCORES):
        b, hg = divmod(c, HG)
        rows = slice(DLOC * hg, DLOC * (hg + 1))
        in_maps.append(
            {
                "xT": _bf16(x[b].T),
                "wqT": _bf16((Wq[rows] * sc).T),
                "wkT": _bf16(Wk[rows].T),
                "wvT": _bf16(Wv[rows].T),
                "woT": _bf16(Wo[:, rows].T),
                "bqT": np.ascontiguousarray((bq[rows] * sc).reshape(HPG, 128).T),
                "bkT": np.ascontiguousarray(bk[rows].reshape(HPG, 128).T),
                "bv": np.ascontiguousarray(
                    np.broadcast_to(bv[rows].reshape(1, DLOC), (128, DLOC))
                ).astype(np.float32),
                **shared,
            }
        )
    return in_maps


def postprocess_y(y):
    return np.asarray(y, np.float32)


def kernel(**inputs) -> np.ndarray:
    from concourse.bass_utils import run_bass_kernel_spmd

    mode = _detect_mode(inputs["attn_mask"])
    nc = _get_program(mode)
    in_maps = build_in_maps(inputs, mode)
    bo = np.asarray(inputs["bo"], np.float32)

    trace = bool(int(os.environ.get("KERNEL_TRACE", "0")))
    try:
        res = run_bass_kernel_spmd(nc, in_maps, list(range(NCORES)), trace=trace)
    except ModuleNotFoundError:
        # environments without the NTFF profile hook can still execute
        trace = False
        res = run_bass_kernel_spmd(nc, in_maps, list(range(NCORES)), trace=False)
    if trace and res.exec_time_ns is not None:
        print(f"HW exec time: {res.exec_time_ns} ns")
        globals()["LAST_EXEC_NS"] = res.exec_time_ns
        globals()["LAST_RESULTS"] = res

    out = np.zeros((B, S, H), np.float32)
    for c in range(NCORES):
        out[c // HG] += postprocess_y(res.results[c]["y"])
    out += bo
    return out


# revision 53
# speedup vs baseline: 1.0244x; 1.0244x over previous
"""Trainium2 Bass kernel: causal multi-head attention block (B=2,S=2048,H=2048,NH=16,HD=128).

Sharding: 8 cores = DP over batch (2) x TP over head-groups (4 groups of 4 heads).
Each core computes q/k/v projections for its 4 heads, RoPE, causal softmax
attention, and a partial output projection; the host sums the 4 partials per
batch and adds bo.

Device layouts (all chosen so every matmul streams natural contiguous-free
tiles; the host pre-transposes x and the weights and casts matmul operands to
bf16 -- accumulation stays fp32 in PSUM):
  xT   [H=2048(f), S=2048(s)]   = x[b].T                       bf16
  wqT  [2048(f), 512(d)]        = (Wq[rows]/sqrt(HD)).T        bf16
  wkT  [2048(f), 512(d)]        = Wk[rows].T                   bf16
  wvT  [2048(f), 512(d)]        = Wv[rows].T                   bf16
  woT  [512(d), 2048(o)]        = Wo[:, rows].T                bf16
Attention runs with transposed score tiles ST[k,q] so the P@V matmul needs no
on-chip transposes; the softmax denominators come from an all-ones [128,128]
stationary matmul, which lands the row sums broadcast across every PSUM
partition so 1/denom is a single DVE reciprocal.

Schedule notes:
 - x is DMA'd once into a persistent 8 MiB SBUF cache ([128, ft, s] per
   512-wide s-block) and reused by both the Q/K and the V projections.
 - each dma_start occupies its issuing sequencer for ~2.2us + transfer time,
   so DMAs are split across two queues (SP/HWDGE for weights+y, Pool/SWDGE
   for x+y) and kept few and large, chunked only where startup pacing needs.
 - RoPE's rotate_half runs on the DVE as a partition pair-swap stream_shuffle
   with the sign folded into the host-built sin table (no PE matmul).
 - Q/K projection accumulators are processed head-PAIR-major so PSUM banks
   drain (bias+RoPE) while the next pair's matmuls stream.
 - the attention inner loop keeps a 4-deep score-matmul pipeline (2 PSUM
   banks from the score pool + 2 borrowed from the idle out-proj pool), so
   the PE never waits on the mask-add + Exp chain.
 - output-projection PSUM is drained by the Scalar engine (DVE is busier)
   and y is stored/DMA'd as bf16; the host accumulates partials in f32.
"""

import math
import os
import sys

import numpy as np

for _p in ("/opt/trn_rl_repo",):
    if _p not in sys.path and os.path.isdir(_p):
        sys.path.insert(0, _p)

import ml_dtypes

import concourse.bass as bass
import concourse.mybir as mybir
import concourse.tile as tile
from concourse import bacc

B, S, H, NH, HD = 2, 2048, 2048, 16, 128
NCORES = 8
HG = 4            # head-groups (TP degree)
HPG = NH // HG    # heads per group = 4
DLOC = HPG * HD   # local d width = 512
FT = H // 128     # 16 f-tiles
SJ = S // 512     # 4 s/q tiles of 512
KT128 = S // 128  # 16 k-tiles of 128
NEG = -1e30
WCH = 4           # f-tiles per startup DMA chunk (512 KiB)

F32 = mybir.dt.float32
F16 = mybir.dt.float16
BF16 = mybir.dt.bfloat16
NPBF16 = ml_dtypes.bfloat16
YDT = BF16        # on-device y dtype (host accumulates in f32)
NPYDT = NPBF16


def build_program(mode: str) -> bass.Bass:
    """mode in {'causal', 'full', 'bias'}"""
    # Collapse Tile's DMA-completion sem round-robin to one lane per DGE class.
    # SP-issued HWDGE DMAs retire FIFO, so a single counting sem stays sound,
    # and every consumer's DMA waits merge into one sem -- without this, waits
    # on 3+ distinct DMAHW*/DMASW* sems overflow the ISA sync-wait budget on
    # DMA instructions ("Too many sync wait commands" in walrus codegen).
    import concourse.tile_sem_assignment as tsa

    tsa.NUM_HWDGE_SEMS = 1
    tsa.NUM_SWDGE_GLOBAL_SEMS = 1
    nc = bacc.Bacc()
    xT = nc.dram_tensor("xT", [H, S], BF16, kind="ExternalInput")
    wqT = nc.dram_tensor("wqT", [H, DLOC], BF16, kind="ExternalInput")
    wkT = nc.dram_tensor("wkT", [H, DLOC], BF16, kind="ExternalInput")
    wvT = nc.dram_tensor("wvT", [H, DLOC], BF16, kind="ExternalInput")
    woT = nc.dram_tensor("woT", [DLOC, H], BF16, kind="ExternalInput")
    bqT = nc.dram_tensor("bqT", [128, HPG], F32, kind="ExternalInput")
    bkT = nc.dram_tensor("bkT", [128, HPG], F32, kind="ExternalInput")
    bv = nc.dram_tensor("bv", [128, DLOC], F32, kind="ExternalInput")
    cosT = nc.dram_tensor("cosT", [HD, S], BF16, kind="ExternalInput")
    # sinT is sign-folded on the host: row 2i holds -sin, row 2i+1 holds +sin,
    # so RoPE's rotate_half is a partition pair-swap (DVE stream_shuffle).
    sinT = nc.dram_tensor("sinT", [HD, S], BF16, kind="ExternalInput")
    if mode == "causal":
        dbias = nc.dram_tensor("dbias", [4, 128, 512], F32, kind="ExternalInput")
    elif mode == "bias":
        fbias = nc.dram_tensor("fbias", [S, S], F32, kind="ExternalInput")
    y = nc.dram_tensor("y", [S, H], YDT, kind="ExternalOutput")

    with tile.TileContext(nc) as tc:
        with (
            tc.tile_pool(name="qt", bufs=HPG * SJ) as qt_pool,
            tc.tile_pool(name="kt", bufs=HPG * SJ) as kt_pool,
            tc.tile_pool(name="vt", bufs=KT128) as vt_pool,
            tc.tile_pool(name="xin", bufs=SJ) as xin_pool,
            tc.tile_pool(name="consts", bufs=1) as consts,
        ):
            QT = {}  # (h, sj) -> [128, 512] bf16 tile, RoPE'd q^T (pre-scaled)
            KT = {}  # (h, sj) -> [128, 512] bf16 tile, RoPE'd k^T
            VT = {}  # ssub -> [128(s), 512(d)] bf16 tile, v + bias

            ones_sb = consts.tile([128, 128], BF16, tag="ones")
            warm_sb = consts.tile([1, 1], F32, tag="warm")
            bv_sb = consts.tile([128, DLOC], F32, tag="bv")
            wo_sb = consts.tile([128, HPG, H], BF16, tag="wo")
            db_sb = None
            if mode == "causal":
                db_sb = consts.tile([128, 4, 512], F32, tag="db")

            # persistent x cache: [128, ft, s-block] per 512-wide s-block
            xts = [
                xin_pool.tile([128, FT, 512], BF16, tag="xt", name="xt")
                for _ in range(SJ)
            ]

            # ================= Phase 1: Q/K projections + RoPE =================
            with (
                tc.tile_pool(name="ps12", bufs=8, space="PSUM") as psum,
                tc.tile_pool(name="wqk", bufs=1) as wqk_pool,
                tc.tile_pool(name="csn", bufs=1) as csn_pool,
                tc.tile_pool(name="rtmp", bufs=4) as rtmp_pool,
            ):
                wq_sb = wqk_pool.tile([128, FT, DLOC], BF16, tag="wq")
                wk_sb = wqk_pool.tile([128, FT, DLOC], BF16, tag="wk")
                cos_sb = csn_pool.tile([HD, S], BF16, tag="cos")
                sin_sb = csn_pool.tile([HD, S], BF16, tag="sin")
                bq_sb = consts.tile([128, HPG], F32, tag="bq")
                bk_sb = consts.tile([128, HPG], F32, tag="bk")

                # Two DMA issue queues (SP + Pool); transfers all serialize
                # on the shared ~360GB/s DMA path, so chunks are interleaved
                # in PE consumption order and kept small only at startup.
                def wchunk(eng, dst, src, f0, n):
                    eng.dma_start(
                        dst[:, f0 : f0 + n, :],
                        src[f0 * 128 : (f0 + n) * 128, :].rearrange(
                            "(ft p) d -> p ft d", p=128
                        ),
                    )

                def xchunk(eng, sj, f0, n):
                    eng.dma_start(
                        xts[sj][:, f0 : f0 + n, :],
                        xT[
                            f0 * 128 : (f0 + n) * 128,
                            sj * 512 : (sj + 1) * 512,
                        ].rearrange("(ft p) s -> p ft s", p=128),
                    )

                nc.vector.memset(ones_sb[:], 1.0)
                # warmup: pull the ACT function-table load into the startup
                # DMA shadow (Exp's set also contains Identity/Copy)
                nc.scalar.activation(
                    warm_sb[:], ones_sb[0:1, 0:1],
                    mybir.ActivationFunctionType.Exp,
                )
                # PE warmup in the startup DMA shadow: ~3.4us of matmul
                # activity releases the HAM clock gate before real work lands
                warm_ps = psum.tile([128, 128], F32, tag="ps", name="warm_ps")
                for _ in range(30):
                    nc.tensor.matmul(
                        warm_ps[:], ones_sb[:], ones_sb[:], start=True, stop=True
                    )
                # Pool queue: x first (paces the first q matmuls), then the
                # first half of wk (k consumption starts at ~19us)
                xchunk(nc.gpsimd, 0, 0, 4)
                xchunk(nc.gpsimd, 0, 4, 4)
                xchunk(nc.gpsimd, 0, 8, FT // 2)
                wchunk(nc.gpsimd, wk_sb, wkT, 0, FT // 2)
                for sj in range(1, SJ):
                    xchunk(nc.gpsimd, sj, 0, FT)
                # SP queue: wq pieces, second wk half, small consts
                wchunk(nc.sync, wq_sb, wqT, 0, 4)
                wchunk(nc.sync, wq_sb, wqT, 4, 4)
                wchunk(nc.sync, wq_sb, wqT, 8, FT // 2)
                wchunk(nc.sync, wk_sb, wkT, FT // 2, FT // 2)
                nc.sync.dma_start(bq_sb[:], bqT[:])
                nc.sync.dma_start(bk_sb[:], bkT[:])
                nc.sync.dma_start(cos_sb[:], cosT[:])
                nc.sync.dma_start(sin_sb[:], sinT[:])

                # head-PAIR-major: a pair's two PSUM accumulators complete
                # every 32 matmuls, so bias/RoPE for one pair pipeline under
                # the next pair's matmuls (full ft-major would serialize all
                # 8 ACT identities at the block end; full h-major starves the
                # startup DMA pacing).
                # RoPE rotate_half = swap adjacent partition pairs (the sign
                # lives in the host-folded sin table)
                swap_mask = [p ^ 1 for p in range(32)]

                def rope(ps, h, bias_sb, store, sj, pool, css, sss):
                    t = pool.tile([128, 512], BF16, tag="t", name="qkt")
                    # t = bf16(psum + per-partition bias); frees the bank
                    nc.scalar.activation(
                        t[:],
                        ps[:],
                        mybir.ActivationFunctionType.Identity,
                        bias=bias_sb[:, h : h + 1],
                    )
                    sh = rtmp_pool.tile([128, 512], BF16, tag="sh", name="sh")
                    nc.vector.stream_shuffle(sh[:], t[:], swap_mask)
                    tmp = rtmp_pool.tile([128, 512], BF16, tag="tmp", name="tmp")
                    nc.vector.tensor_mul(tmp[:], sh[:], sss)
                    nc.vector.tensor_mul(t[:], t[:], css)
                    nc.vector.tensor_add(t[:], t[:], tmp[:])
                    store[(h, sj)] = t

                for sj in range(SJ):
                    xt = xts[sj]
                    css = cos_sb[:, sj * 512 : (sj + 1) * 512]
                    sss = sin_sb[:, sj * 512 : (sj + 1) * 512]
                    for which, w_sb, bias_sb, store, pool in (
                        ("q", wq_sb, bq_sb, QT, qt_pool),
                        ("k", wk_sb, bk_sb, KT, kt_pool),
                    ):
                        for hp in range(HPG // 2):
                            pair = (2 * hp, 2 * hp + 1)
                            pp = {
                                h: psum.tile([128, 512], F32, tag="ps", name="ps")
                                for h in pair
                            }
                            for ft in range(FT):
                                for h in pair:
                                    nc.tensor.matmul(
                                        pp[h][:],
                                        w_sb[:, ft, h * 128 : (h + 1) * 128],
                                        xt[:, ft, :],
                                        start=(ft == 0),
                                        stop=(ft == FT - 1),
                                    )
                            for h in pair:
                                rope(pp[h], h, bias_sb, store, sj, pool, css, sss)

                # ================= Phase 2: V projection =================
                # reuses the resident x cache -- no x reload.
                with tc.tile_pool(name="wv", bufs=1) as wv_pool:
                    wv_sb = wv_pool.tile([128, FT, DLOC], BF16, tag="wv")
                    nc.sync.dma_start(
                        wv_sb[:], wvT.rearrange("(ft p) d -> p ft d", p=128)
                    )
                    nc.sync.dma_start(bv_sb[:], bv[:])
                    # phase-3 constants ride behind phase-2's weights on the FIFO
                    if mode == "causal":
                        nc.sync.dma_start(
                            db_sb[:], dbias.rearrange("a p t -> p a t")
                        )
                    nc.sync.dma_start(
                        wo_sb[:], woT.rearrange("(dt p) o -> p dt o", p=128)
                    )
                    for ss in range(KT128):
                        sj, cc = divmod(ss, 4)
                        vp = psum.tile([128, 512], F32, tag="ps", name="ps")
                        for ft in range(FT):
                            nc.tensor.matmul(
                                vp[:],
                                xts[sj][:, ft, cc * 128 : (cc + 1) * 128],
                                wv_sb[:, ft, :],
                                start=(ft == 0),
                                stop=(ft == FT - 1),
                            )
                        v = vt_pool.tile([128, DLOC], BF16, tag="v", name="v")
                        nc.vector.tensor_add(v[:], vp[:], bv_sb[:])
                        VT[ss] = v

            # ============ Phase 3: attention + output projection ============
            with (
                tc.tile_pool(name="pst", bufs=2, space="PSUM") as psum_st,
                tc.tile_pool(name="ppv", bufs=2, space="PSUM") as psum_pv,
                tc.tile_pool(name="pdn", bufs=2, space="PSUM") as psum_dn,
                tc.tile_pool(name="pyp", bufs=2, space="PSUM") as psum_yp,
                tc.tile_pool(name="ex", bufs=8) as exp_pool,
                tc.tile_pool(name="ot", bufs=2 * HPG) as ot_pool,
                tc.tile_pool(name="rc", bufs=4) as rc_pool,
                tc.tile_pool(name="ysb", bufs=4) as y_pool,
                tc.tile_pool(name="fb", bufs=3) as fb_pool,
            ):
                def outproj_tile(OTsrc, qj_src, ss, oj, ysbs, copy_eng):
                    # one [128,512] out-proj tile for q-block qj_src; the ysb
                    # row-block DMAs out when its 4th column chunk lands
                    if ss not in ysbs:
                        ysbs[ss] = y_pool.tile([128, H], YDT, tag="y", name="y")
                    ysb = ysbs[ss]
                    yp = psum_yp.tile([128, 512], F32, tag="yp", name="yp")
                    for dt in range(HPG):
                        nc.tensor.matmul(
                            yp[:],
                            OTsrc[dt][:, ss * 128 : (ss + 1) * 128],
                            wo_sb[:, dt, oj * 512 : (oj + 1) * 512],
                            start=(dt == 0),
                            stop=(dt == HPG - 1),
                        )
                    if copy_eng == "dve":
                        nc.vector.tensor_copy(
                            ysb[:, oj * 512 : (oj + 1) * 512], yp[:]
                        )
                    else:
                        nc.scalar.activation(
                            ysb[:, oj * 512 : (oj + 1) * 512],
                            yp[:],
                            mybir.ActivationFunctionType.Copy,
                        )
                    if oj == 3:
                        r0 = qj_src * 512 + ss * 128
                        eng = nc.sync if (qj_src * 4 + ss) % 2 == 0 else nc.gpsimd
                        eng.dma_start(y[r0 : r0 + 128, :], ysb[:])

                def outproj_block(OTsrc, qj_src, tail):
                    # inline projection of a whole q-block (non-interleaved)
                    ysbs = {}
                    for ss in range(4):
                        last_block = tail and ss == 3
                        if last_block:
                            ysb = y_pool.tile([128, H], YDT, tag="y", name="y")
                            ysbs[ss] = ysb
                            for oj in range(4):
                                yp = psum_yp.tile(
                                    [128, 512], F32, tag="yp", name="yp"
                                )
                                for dt in range(HPG):
                                    nc.tensor.matmul(
                                        yp[:],
                                        OTsrc[dt][:, ss * 128 : (ss + 1) * 128],
                                        wo_sb[:, dt, oj * 512 : (oj + 1) * 512],
                                        start=(dt == 0),
                                        stop=(dt == HPG - 1),
                                    )
                                if oj % 2 == 1:
                                    # tail: alternate copies onto DVE so the
                                    # final chain isn't serialized on ACT
                                    nc.vector.tensor_copy(
                                        ysb[:, oj * 512 : (oj + 1) * 512], yp[:]
                                    )
                                else:
                                    nc.scalar.activation(
                                        ysb[:, oj * 512 : (oj + 1) * 512],
                                        yp[:],
                                        mybir.ActivationFunctionType.Copy,
                                    )
                            r0 = qj_src * 512 + ss * 128
                            # split the final DMA across both queues so the
                            # two issue latencies overlap
                            nc.sync.dma_start(
                                y[r0 : r0 + 128, : H // 2], ysb[:, : H // 2]
                            )
                            nc.gpsimd.dma_start(
                                y[r0 : r0 + 128, H // 2 :], ysb[:, H // 2 :]
                            )
                        else:
                            for oj in range(4):
                                outproj_tile(OTsrc, qj_src, ss, oj, ysbs, "act")

                OTprev = None
                for qj in range(SJ):
                    kmax = 4 * qj + 4 if mode == "causal" else KT128
                    seq = [(h, kj) for h in range(HPG) for kj in range(kmax)]
                    ST = {}
                    PV = {}
                    DN = {}
                    OT = {}
                    # the previous q-block's out-projection interleaves into
                    # this block's attention: its PE-only matmul groups slot
                    # in after diagonal tiles, exactly where the Exp stream
                    # otherwise falls behind the PE (copies drain via DVE)
                    pending = (
                        [(ss, oj) for ss in range(4) for oj in range(4)]
                        if (mode == "causal" and OTprev is not None)
                        else []
                    )
                    pend_ysbs = {}

                    def issue_st(i):
                        h, kj = seq[i]
                        # columns of this q-tile that the k-tile can see at
                        # all (causal): the diagonal k-tile only reaches
                        # q >= its own first row.
                        a = kj - 4 * qj
                        off = 128 * a if (mode == "causal" and a > 0) else 0
                        # alternate pools (pyp is shared) so a 2-deep score
                        # pipeline fits the 8 PSUM banks; the borrowed slots
                        # share pyp's "yp" tag/ring
                        if i % 2 == 0:
                            st = psum_st.tile([128, 512], F32, tag="st", name="st")
                        else:
                            st = psum_yp.tile([128, 512], F32, tag="yp", name="st")
                        nc.tensor.matmul(
                            st[:, off:],
                            KT[(h, kj // 4)][:, (kj % 4) * 128 : (kj % 4 + 1) * 128],
                            QT[(h, qj)][:, off:],
                            start=True,
                            stop=True,
                        )
                        ST[i] = (st, off)

                    def normalize(h):
                        # DN[h] holds the denominators broadcast across all
                        # 128 partitions (all-ones stationary), so 1/dn is a
                        # single DVE reciprocal and ot a single multiply.
                        rcb = rc_pool.tile([128, 512], F32, tag="rcb", name="rcb")
                        nc.vector.reciprocal_approx_fast(rcb[:], DN[h][:])
                        ot = ot_pool.tile([128, 512], BF16, tag="ot", name="ot")
                        nc.vector.tensor_mul(ot[:], PV[h][:], rcb[:])
                        OT[h] = ot

                    DEPTH = 4  # st ring slots: 2 in pst + 2 borrowed from pyp
                    for i in range(min(DEPTH, len(seq))):
                        issue_st(i)
                    for i, (h, kj) in enumerate(seq):
                        st, off = ST.pop(i)
                        a = kj - 4 * qj
                        if mode == "causal" and a >= 0:
                            # only the 128-wide band straddling the diagonal
                            # is partially masked
                            nc.vector.tensor_add(
                                st[:, off : off + 128],
                                st[:, off : off + 128],
                                db_sb[:, a, off : off + 128],
                            )
                        elif mode == "bias":
                            fb = fb_pool.tile([128, 512], F32, tag="fb", name="fb")
                            nc.sync.dma_start(
                                fb[:],
                                fbias[
                                    kj * 128 : (kj + 1) * 128,
                                    qj * 512 : (qj + 1) * 512,
                                ],
                            )
                            nc.vector.tensor_add(st[:], st[:], fb[:])
                        e = exp_pool.tile([128, 512], BF16, tag="e", name="e")
                        nc.scalar.activation(
                            e[:, off:], st[:, off:],
                            mybir.ActivationFunctionType.Exp,
                        )
                        if kj == 0:
                            PV[h] = psum_pv.tile([128, 512], F32, tag="pv", name="pv")
                            DN[h] = psum_dn.tile([128, 512], F32, tag="dn", name="dn")
                        nc.tensor.matmul(
                            PV[h][:, off:],
                            VT[kj][:, h * 128 : (h + 1) * 128],
                            e[:, off:],
                            start=(kj == 0),
                            stop=(kj == kmax - 1),
                        )
                        nc.tensor.matmul(
                            DN[h][:, off:],
                            ones_sb[:],
                            e[:, off:],
                            start=(kj == 0),
                            stop=(kj == kmax - 1),
                        )
                        if i + DEPTH < len(seq):
                            issue_st(i + DEPTH)
                        if pending and a >= 0:
                            ss, oj = pending.pop(0)
                            outproj_tile(OTprev, qj - 1, ss, oj, pend_ysbs, "dve")
                        if kj == kmax - 1:
                            normalize(h)
                    for ss, oj in pending:
                        outproj_tile(OTprev, qj - 1, ss, oj, pend_ysbs, "act")
                    if mode == "causal":
                        if qj == SJ - 1:
                            outproj_block(OT, qj, tail=True)
                    else:
                        outproj_block(OT, qj, tail=(qj == SJ - 1))
                    OTprev = OT
    nc.compile()
    return nc


_PROGRAM_CACHE = {}


def _get_program(mode):
    if mode not in _PROGRAM_CACHE:
        _PROGRAM_CACHE[mode] = build_program(mode)
    return _PROGRAM_CACHE[mode]


def _detect_mode(attn_mask):
    m = np.asarray(attn_mask).reshape(S, S)
    if (m == np.tril(np.ones((S, S), m.dtype))).all():
        return "causal"
    if (m != 0).all():
        return "full"
    return "bias"


def _diag_bias():
    a = np.arange(4)[:, None, None]
    p = np.arange(128)[None, :, None]
    t = np.arange(512)[None, None, :]
    return np.where(128 * a + p <= t, 0.0, NEG).astype(np.float32)


def _bf16(a):
    return np.ascontiguousarray(a).astype(NPBF16)


def build_in_maps(inputs, mode):
    x = np.asarray(inputs["x"], np.float32)
    fcos = np.asarray(inputs["fcos"], np.float32)
    fsin = np.asarray(inputs["fsin"], np.float32)
    Wq, bq = np.asarray(inputs["Wq"], np.float32), np.asarray(inputs["bq"], np.float32)
    Wk, bk = np.asarray(inputs["Wk"], np.float32), np.asarray(inputs["bk"], np.float32)
    Wv, bv = np.asarray(inputs["Wv"], np.float32), np.asarray(inputs["bv"], np.float32)
    Wo = np.asarray(inputs["Wo"], np.float32)
    attn_mask = inputs["attn_mask"]

    sc = 1.0 / math.sqrt(HD)
    sgn = np.where(np.arange(HD) % 2 == 0, -1.0, 1.0).astype(np.float32)[:, None]
    shared = {
        "cosT": _bf16(fcos.T),
        "sinT": _bf16(fsin.T * sgn),
    }
    if mode == "causal":
        shared["dbias"] = _diag_bias()
    elif mode == "bias":
        m = np.asarray(attn_mask).reshape(S, S)
        shared["fbias"] = np.ascontiguousarray(
            np.where(m.T == 0, NEG, 0.0).astype(np.float32)
        )

    in_maps = []
    for c in range(NCORES):
        b, hg = divmod(c, HG)
        rows = slice(DLOC * hg, DLOC * (hg + 1))
        in_maps.append(
            {
                "xT": _bf16(x[b].T),
                "wqT": _bf16((Wq[rows] * sc).T),
                "wkT": _bf16(Wk[rows].T),
                "wvT": _bf16(Wv[rows].T),
                "woT": _bf16(Wo[:, rows].T),
                "bqT": np.ascontiguousarray((bq[rows] * sc).reshape(HPG, 128).T),
                "bkT": np.ascontiguousarray(bk[rows].reshape(HPG, 128).T),
                "bv": np.ascontiguousarray(
                    np.broadcast_to(bv[rows].reshape(1, DLOC), (128, DLOC))
                ).astype(np.float32),
                **shared,
            }
        )
    return in_maps


def postprocess_y(y):
    return np.asarray(y, np.float32)


def kernel(**inputs) -> np.ndarray:
    from concourse.bass_utils import run_bass_kernel_spmd

    mode = _detect_mode(inputs["attn_mask"])
    nc = _get_program(mode)
    in_maps = build_in_maps(inputs, mode)
    bo = np.asarray(inputs["bo"], np.float32)

    trace = bool(int(os.environ.get("KERNEL_TRACE", "0")))
    try:
        res = run_bass_kernel_spmd(nc, in_maps, list(range(NCORES)), trace=trace)
    except ModuleNotFoundError:
        # environments without the NTFF profile hook can still execute
        trace = False
        res = run_bass_kernel_spmd(nc, in_maps, list(range(NCORES)), trace=False)
    if trace and res.exec_time_ns is not None:
        print(f"HW exec time: {res.exec_time_ns} ns")
        globals()["LAST_EXEC_NS"] = res.exec_time_ns
        globals()["LAST_RESULTS"] = res

    out = np.zeros((B, S, H), np.float32)
    for c in range(NCORES):
        out[c // HG] += postprocess_y(res.results[c]["y"])
    out += bo
    return out


# revision 63
# speedup vs baseline: 1.0304x; 1.0059x over previous
"""Trainium2 Bass kernel: causal multi-head attention block (B=2,S=2048,H=2048,NH=16,HD=128).

Sharding: 8 cores = DP over batch (2) x TP over head-groups (4 groups of 4 heads).
Each core computes q/k/v projections for its 4 heads, RoPE, causal softmax
attention, and a partial output projection; the host sums the 4 partials per
batch and adds bo.

Device layouts (all chosen so every matmul streams natural contiguous-free
tiles; the host pre-transposes x and the weights and casts matmul operands to
bf16 -- accumulation stays fp32 in PSUM):
  xT   [H=2048(f), S=2048(s)]   = x[b].T                       bf16
  wqT  [2048(f), 512(d)]        = (Wq[rows]/sqrt(HD)).T        bf16
  wkT  [2048(f), 512(d)]        = Wk[rows].T                   bf16
  wvT  [2048(f), 512(d)]        = Wv[rows].T                   bf16
  woT  [512(d), 2048(o)]        = Wo[:, rows].T                bf16
Attention runs with transposed score tiles ST[k,q] so the P@V matmul needs no
on-chip transposes; the softmax denominators come from an all-ones [128,128]
stationary matmul, which lands the row sums broadcast across every PSUM
partition so 1/denom is a single DVE reciprocal.

Schedule notes:
 - x is DMA'd once into a persistent 8 MiB SBUF cache ([128, ft, s] per
   512-wide s-block) and reused by both the Q/K and the V projections.
 - each dma_start occupies its issuing sequencer for ~2.2us + transfer time,
   so DMAs are split across two queues (SP/HWDGE for weights+y, Pool/SWDGE
   for x+y) and kept few and large, chunked only where startup pacing needs.
 - RoPE's rotate_half runs on the DVE as a partition pair-swap stream_shuffle
   with the sign folded into the host-built sin table (no PE matmul).
 - Q/K projection accumulators are processed head-PAIR-major so PSUM banks
   drain (bias+RoPE) while the next pair's matmuls stream.
 - the attention inner loop keeps a 4-deep score-matmul pipeline (2 PSUM
   banks from the score pool + 2 borrowed from the out-proj pool), so the
   PE never waits on the mask-add + Exp chain.
 - each q-block's output projection interleaves into the NEXT block's
   attention (one PE-only yp tile after each diagonal iteration, copies on
   the DVE), plugging the spots where the Exp stream lags the PE; only the
   last block projects inline, with its copies split ACT/DVE and its final
   DMA split across both issue queues to shorten the drain tail.
 - ~30 dummy matmuls in the startup DMA shadow release the HAM clock gate
   so the first real matmuls run at full clock.
 - y is stored/DMA'd as bf16; the host accumulates partials in f32.
"""

import math
import os
import sys

import numpy as np

for _p in ("/opt/trn_rl_repo",):
    if _p not in sys.path and os.path.isdir(_p):
        sys.path.insert(0, _p)

import ml_dtypes

import concourse.bass as bass
import concourse.mybir as mybir
import concourse.tile as tile
from concourse import bacc

B, S, H, NH, HD = 2, 2048, 2048, 16, 128
NCORES = 8
HG = 4            # head-groups (TP degree)
HPG = NH // HG    # heads per group = 4
DLOC = HPG * HD   # local d width = 512
FT = H // 128     # 16 f-tiles
SJ = S // 512     # 4 s/q tiles of 512
KT128 = S // 128  # 16 k-tiles of 128
NEG = -1e30
WCH = 4           # f-tiles per startup DMA chunk (512 KiB)

F32 = mybir.dt.float32
F16 = mybir.dt.float16
BF16 = mybir.dt.bfloat16
NPBF16 = ml_dtypes.bfloat16
YDT = BF16        # on-device y dtype (host accumulates in f32)
NPYDT = NPBF16


def build_program(mode: str) -> bass.Bass:
    """mode in {'causal', 'full', 'bias'}"""
    # Collapse Tile's DMA-completion sem round-robin to one lane per DGE class.
    # SP-issued HWDGE DMAs retire FIFO, so a single counting sem stays sound,
    # and every consumer's DMA waits merge into one sem -- without this, waits
    # on 3+ distinct DMAHW*/DMASW* sems overflow the ISA sync-wait budget on
    # DMA instructions ("Too many sync wait commands" in walrus codegen).
    import concourse.tile_sem_assignment as tsa

    tsa.NUM_HWDGE_SEMS = 1
    tsa.NUM_SWDGE_GLOBAL_SEMS = 1
    nc = bacc.Bacc()
    xT = nc.dram_tensor("xT", [H, S], BF16, kind="ExternalInput")
    wqT = nc.dram_tensor("wqT", [H, DLOC], BF16, kind="ExternalInput")
    wkT = nc.dram_tensor("wkT", [H, DLOC], BF16, kind="ExternalInput")
    wvT = nc.dram_tensor("wvT", [H, DLOC], BF16, kind="ExternalInput")
    woT = nc.dram_tensor("woT", [DLOC, H], BF16, kind="ExternalInput")
    bqT = nc.dram_tensor("bqT", [128, HPG], F32, kind="ExternalInput")
    bkT = nc.dram_tensor("bkT", [128, HPG], F32, kind="ExternalInput")
    bv = nc.dram_tensor("bv", [128, DLOC], F32, kind="ExternalInput")
    cosT = nc.dram_tensor("cosT", [HD, S], BF16, kind="ExternalInput")
    # sinT is sign-folded on the host: row 2i holds -sin, row 2i+1 holds +sin,
    # so RoPE's rotate_half is a partition pair-swap (DVE stream_shuffle).
    sinT = nc.dram_tensor("sinT", [HD, S], BF16, kind="ExternalInput")
    if mode == "causal":
        dbias = nc.dram_tensor("dbias", [4, 128, 512], F32, kind="ExternalInput")
    elif mode == "bias":
        fbias = nc.dram_tensor("fbias", [S, S], F32, kind="ExternalInput")
    y = nc.dram_tensor("y", [S, H], YDT, kind="ExternalOutput")

    with tile.TileContext(nc) as tc:
        with (
            tc.tile_pool(name="qt", bufs=HPG * SJ) as qt_pool,
            tc.tile_pool(name="kt", bufs=HPG * SJ) as kt_pool,
            tc.tile_pool(name="vt", bufs=KT128) as vt_pool,
            tc.tile_pool(name="xin", bufs=SJ) as xin_pool,
            tc.tile_pool(name="consts", bufs=1) as consts,
        ):
            QT = {}  # (h, sj) -> [128, 512] bf16 tile, RoPE'd q^T (pre-scaled)
            KT = {}  # (h, sj) -> [128, 512] bf16 tile, RoPE'd k^T
            VT = {}  # ssub -> [128(s), 512(d)] bf16 tile, v + bias

            ones_sb = consts.tile([128, 128], BF16, tag="ones")
            warm_sb = consts.tile([1, 1], F32, tag="warm")
            bv_sb = consts.tile([128, DLOC], F32, tag="bv")
            wo_sb = consts.tile([128, HPG, H], BF16, tag="wo")
            db_sb = None
            if mode == "causal":
                db_sb = consts.tile([128, 4, 512], F32, tag="db")

            # persistent x cache: [128, ft, s-block] per 512-wide s-block
            xts = [
                xin_pool.tile([128, FT, 512], BF16, tag="xt", name="xt")
                for _ in range(SJ)
            ]

            # ================= Phase 1: Q/K projections + RoPE =================
            with (
                tc.tile_pool(name="ps12", bufs=8, space="PSUM") as psum,
                tc.tile_pool(name="wqk", bufs=1) as wqk_pool,
                tc.tile_pool(name="csn", bufs=1) as csn_pool,
                tc.tile_pool(name="rtmp", bufs=4) as rtmp_pool,
            ):
                wq_sb = wqk_pool.tile([128, FT, DLOC], BF16, tag="wq")
                wk_sb = wqk_pool.tile([128, FT, DLOC], BF16, tag="wk")
                cos_sb = csn_pool.tile([HD, S], BF16, tag="cos")
                sin_sb = csn_pool.tile([HD, S], BF16, tag="sin")
                bq_sb = consts.tile([128, HPG], F32, tag="bq")
                bk_sb = consts.tile([128, HPG], F32, tag="bk")

                # Two DMA issue queues (SP + Pool); transfers all serialize
                # on the shared ~360GB/s DMA path, so chunks are interleaved
                # in PE consumption order and kept small only at startup.
                def wchunk(eng, dst, src, f0, n):
                    eng.dma_start(
                        dst[:, f0 : f0 + n, :],
                        src[f0 * 128 : (f0 + n) * 128, :].rearrange(
                            "(ft p) d -> p ft d", p=128
                        ),
                    )

                def xchunk(eng, sj, f0, n):
                    eng.dma_start(
                        xts[sj][:, f0 : f0 + n, :],
                        xT[
                            f0 * 128 : (f0 + n) * 128,
                            sj * 512 : (sj + 1) * 512,
                        ].rearrange("(ft p) s -> p ft s", p=128),
                    )

                nc.vector.memset(ones_sb[:], 1.0)
                # warmup: pull the ACT function-table load into the startup
                # DMA shadow (Exp's set also contains Identity/Copy)
                nc.scalar.activation(
                    warm_sb[:], ones_sb[0:1, 0:1],
                    mybir.ActivationFunctionType.Exp,
                )
                # PE warmup in the startup DMA shadow: ~3.4us of matmul
                # activity releases the HAM clock gate before real work lands
                warm_ps = psum.tile([128, 128], F32, tag="ps", name="warm_ps")
                for _ in range(30):
                    nc.tensor.matmul(
                        warm_ps[:], ones_sb[:], ones_sb[:], start=True, stop=True
                    )
                # Pool queue: x first (paces the first q matmuls), then the
                # first half of wk (k consumption starts at ~19us)
                xchunk(nc.gpsimd, 0, 0, 4)
                xchunk(nc.gpsimd, 0, 4, 4)
                xchunk(nc.gpsimd, 0, 8, FT // 2)
                wchunk(nc.gpsimd, wk_sb, wkT, 0, FT // 2)
                for sj in range(1, SJ):
                    xchunk(nc.gpsimd, sj, 0, FT)
                # SP queue: wq pieces, second wk half, small consts
                wchunk(nc.sync, wq_sb, wqT, 0, 4)
                wchunk(nc.sync, wq_sb, wqT, 4, 4)
                wchunk(nc.sync, wq_sb, wqT, 8, FT // 2)
                wchunk(nc.sync, wk_sb, wkT, FT // 2, FT // 2)
                nc.sync.dma_start(bq_sb[:], bqT[:])
                nc.sync.dma_start(bk_sb[:], bkT[:])
                nc.sync.dma_start(cos_sb[:], cosT[:])
                nc.sync.dma_start(sin_sb[:], sinT[:])

                # ft-major over all 4 heads: consumes each weight/x chunk at
                # 3.4us per 512KiB pair (>= the serial DMA's 2.9us supply
                # rate, so the startup never starves), and the 4+4 PSUM
                # accumulators exactly fill the 8 banks now that RoPE's
                # rotate (DVE shuffle) needs none.
                # RoPE rotate_half = swap adjacent partition pairs (the sign
                # lives in the host-folded sin table)
                swap_mask = [p ^ 1 for p in range(32)]

                def rope(ps, h, bias_sb, store, sj, pool, css, sss):
                    t = pool.tile([128, 512], BF16, tag="t", name="qkt")
                    # t = bf16(psum + per-partition bias); frees the bank
                    nc.scalar.activation(
                        t[:],
                        ps[:],
                        mybir.ActivationFunctionType.Identity,
                        bias=bias_sb[:, h : h + 1],
                    )
                    sh = rtmp_pool.tile([128, 512], BF16, tag="sh", name="sh")
                    nc.vector.stream_shuffle(sh[:], t[:], swap_mask)
                    tmp = rtmp_pool.tile([128, 512], BF16, tag="tmp", name="tmp")
                    nc.vector.tensor_mul(tmp[:], sh[:], sss)
                    nc.vector.tensor_mul(t[:], t[:], css)
                    nc.vector.tensor_add(t[:], t[:], tmp[:])
                    store[(h, sj)] = t

                for sj in range(SJ):
                    xt = xts[sj]
                    css = cos_sb[:, sj * 512 : (sj + 1) * 512]
                    sss = sin_sb[:, sj * 512 : (sj + 1) * 512]
                    for which, w_sb, bias_sb, store, pool in (
                        ("q", wq_sb, bq_sb, QT, qt_pool),
                        ("k", wk_sb, bk_sb, KT, kt_pool),
                    ):
                        pp = {
                            h: psum.tile([128, 512], F32, tag="ps", name="ps")
                            for h in range(HPG)
                        }
                        for ft in range(FT):
                            for h in range(HPG):
                                nc.tensor.matmul(
                                    pp[h][:],
                                    w_sb[:, ft, h * 128 : (h + 1) * 128],
                                    xt[:, ft, :],
                                    start=(ft == 0),
                                    stop=(ft == FT - 1),
                                )
                        for h in range(HPG):
                            rope(pp[h], h, bias_sb, store, sj, pool, css, sss)

                # ================= Phase 2: V projection =================
                # reuses the resident x cache -- no x reload.
                with tc.tile_pool(name="wv", bufs=1) as wv_pool:
                    wv_sb = wv_pool.tile([128, FT, DLOC], BF16, tag="wv")
                    nc.sync.dma_start(
                        wv_sb[:], wvT.rearrange("(ft p) d -> p ft d", p=128)
                    )
                    nc.sync.dma_start(bv_sb[:], bv[:])
                    # phase-3 constants ride behind phase-2's weights on the FIFO
                    if mode == "causal":
                        nc.sync.dma_start(
                            db_sb[:], dbias.rearrange("a p t -> p a t")
                        )
                    nc.sync.dma_start(
                        wo_sb[:], woT.rearrange("(dt p) o -> p dt o", p=128)
                    )
                    for ss in range(KT128):
                        sj, cc = divmod(ss, 4)
                        vp = psum.tile([128, 512], F32, tag="ps", name="ps")
                        for ft in range(FT):
                            nc.tensor.matmul(
                                vp[:],
                                xts[sj][:, ft, cc * 128 : (cc + 1) * 128],
                                wv_sb[:, ft, :],
                                start=(ft == 0),
                                stop=(ft == FT - 1),
                            )
                        v = vt_pool.tile([128, DLOC], BF16, tag="v", name="v")
                        nc.vector.tensor_add(v[:], vp[:], bv_sb[:])
                        VT[ss] = v

            # ============ Phase 3: attention + output projection ============
            with (
                tc.tile_pool(name="pst", bufs=2, space="PSUM") as psum_st,
                tc.tile_pool(name="ppv", bufs=2, space="PSUM") as psum_pv,
                tc.tile_pool(name="pdn", bufs=2, space="PSUM") as psum_dn,
                tc.tile_pool(name="pyp", bufs=2, space="PSUM") as psum_yp,
                tc.tile_pool(name="ex", bufs=12) as exp_pool,
                tc.tile_pool(name="ot", bufs=2 * HPG) as ot_pool,
                tc.tile_pool(name="rc", bufs=4) as rc_pool,
                tc.tile_pool(name="ysb", bufs=4) as y_pool,
                tc.tile_pool(name="fb", bufs=3) as fb_pool,
            ):
                def outproj_tile(OTsrc, qj_src, ss, oj, ysbs, copy_eng):
                    # one [128,512] out-proj tile for q-block qj_src; the ysb
                    # row-block DMAs out when its 4th column chunk lands
                    if ss not in ysbs:
                        ysbs[ss] = y_pool.tile([128, H], YDT, tag="y", name="y")
                    ysb = ysbs[ss]
                    yp = psum_yp.tile([128, 512], F32, tag="yp", name="yp")
                    for dt in range(HPG):
                        nc.tensor.matmul(
                            yp[:],
                            OTsrc[dt][:, ss * 128 : (ss + 1) * 128],
                            wo_sb[:, dt, oj * 512 : (oj + 1) * 512],
                            start=(dt == 0),
                            stop=(dt == HPG - 1),
                        )
                    if copy_eng == "dve":
                        nc.vector.tensor_copy(
                            ysb[:, oj * 512 : (oj + 1) * 512], yp[:]
                        )
                    else:
                        nc.scalar.activation(
                            ysb[:, oj * 512 : (oj + 1) * 512],
                            yp[:],
                            mybir.ActivationFunctionType.Copy,
                        )
                    if oj == 3:
                        r0 = qj_src * 512 + ss * 128
                        eng = nc.sync if (qj_src * 4 + ss) % 2 == 0 else nc.gpsimd
                        eng.dma_start(y[r0 : r0 + 128, :], ysb[:])

                def outproj_block(OTsrc, qj_src, tail):
                    # inline projection of a whole q-block (non-interleaved)
                    ysbs = {}
                    for ss in range(4):
                        last_block = tail and ss == 3
                        if last_block:
                            ysb = y_pool.tile([128, H], YDT, tag="y", name="y")
                            ysbs[ss] = ysb
                            r0 = qj_src * 512 + ss * 128
                            for oj in range(4):
                                yp = psum_yp.tile(
                                    [128, 512], F32, tag="yp", name="yp"
                                )
                                for dt in range(HPG):
                                    nc.tensor.matmul(
                                        yp[:],
                                        OTsrc[dt][:, ss * 128 : (ss + 1) * 128],
                                        wo_sb[:, dt, oj * 512 : (oj + 1) * 512],
                                        start=(dt == 0),
                                        stop=(dt == HPG - 1),
                                    )
                                if oj % 2 == 1:
                                    # tail: alternate copies onto DVE so the
                                    # final chain isn't serialized on ACT
                                    nc.vector.tensor_copy(
                                        ysb[:, oj * 512 : (oj + 1) * 512], yp[:]
                                    )
                                else:
                                    nc.scalar.activation(
                                        ysb[:, oj * 512 : (oj + 1) * 512],
                                        yp[:],
                                        mybir.ActivationFunctionType.Copy,
                                    )
                                if oj == 1:
                                    # issue the low-half DMA as soon as its
                                    # two chunks land; its ~2.2us of SP issue
                                    # overlaps the remaining copies
                                    nc.sync.dma_start(
                                        y[r0 : r0 + 128, : H // 2],
                                        ysb[:, : H // 2],
                                    )
                            nc.gpsimd.dma_start(
                                y[r0 : r0 + 128, H // 2 :], ysb[:, H // 2 :]
                            )
                        else:
                            for oj in range(4):
                                outproj_tile(OTsrc, qj_src, ss, oj, ysbs, "act")

                OTprev = None
                for qj in range(SJ):
                    kmax = 4 * qj + 4 if mode == "causal" else KT128
                    seq = [(h, kj) for h in range(HPG) for kj in range(kmax)]
                    ST = {}
                    PV = {}
                    DN = {}
                    OT = {}
                    # the previous q-block's out-projection interleaves into
                    # this block's attention: its PE-only matmul groups slot
                    # in after diagonal tiles, exactly where the Exp stream
                    # otherwise falls behind the PE (copies drain via DVE)
                    pending = (
                        [(ss, oj) for ss in range(4) for oj in range(4)]
                        if (mode == "causal" and OTprev is not None)
                        else []
                    )
                    pend_ysbs = {}

                    def issue_st(i):
                        h, kj = seq[i]
                        # columns of this q-tile that the k-tile can see at
                        # all (causal): the diagonal k-tile only reaches
                        # q >= its own first row.
                        a = kj - 4 * qj
                        off = 128 * a if (mode == "causal" and a > 0) else 0
                        # alternate pools (pyp is shared) so a 2-deep score
                        # pipeline fits the 8 PSUM banks; the borrowed slots
                        # share pyp's "yp" tag/ring
                        if i % 2 == 0:
                            st = psum_st.tile([128, 512], F32, tag="st", name="st")
                        else:
                            st = psum_yp.tile([128, 512], F32, tag="yp", name="st")
                        nc.tensor.matmul(
                            st[:, off:],
                            KT[(h, kj // 4)][:, (kj % 4) * 128 : (kj % 4 + 1) * 128],
                            QT[(h, qj)][:, off:],
                            start=True,
                            stop=True,
                        )
                        ST[i] = (st, off)

                    def normalize(h):
                        # DN[h] holds the denominators broadcast across all
                        # 128 partitions (all-ones stationary), so 1/dn is a
                        # single DVE reciprocal and ot a single multiply.
                        rcb = rc_pool.tile([128, 512], F32, tag="rcb", name="rcb")
                        nc.vector.reciprocal_approx_fast(rcb[:], DN[h][:])
                        ot = ot_pool.tile([128, 512], BF16, tag="ot", name="ot")
                        nc.vector.tensor_mul(ot[:], PV[h][:], rcb[:])
                        OT[h] = ot

                    DEPTH = 4  # st ring slots: 2 in pst + 2 borrowed from pyp
                    for i in range(min(DEPTH, len(seq))):
                        issue_st(i)
                    for i, (h, kj) in enumerate(seq):
                        st, off = ST.pop(i)
                        a = kj - 4 * qj
                        if mode == "causal" and a >= 0:
                            # only the 128-wide band straddling the diagonal
                            # is partially masked
                            nc.vector.tensor_add(
                                st[:, off : off + 128],
                                st[:, off : off + 128],
                                db_sb[:, a, off : off + 128],
                            )
                        elif mode == "bias":
                            fb = fb_pool.tile([128, 512], F32, tag="fb", name="fb")
                            nc.sync.dma_start(
                                fb[:],
                                fbias[
                                    kj * 128 : (kj + 1) * 128,
                                    qj * 512 : (qj + 1) * 512,
                                ],
                            )
                            nc.vector.tensor_add(st[:], st[:], fb[:])
                        e = exp_pool.tile([128, 512], BF16, tag="e", name="e")
                        nc.scalar.activation(
                            e[:, off:], st[:, off:],
                            mybir.ActivationFunctionType.Exp,
                        )
                        if kj == 0:
                            PV[h] = psum_pv.tile([128, 512], F32, tag="pv", name="pv")
                            DN[h] = psum_dn.tile([128, 512], F32, tag="dn", name="dn")
                        nc.tensor.matmul(
                            PV[h][:, off:],
                            VT[kj][:, h * 128 : (h + 1) * 128],
                            e[:, off:],
                            start=(kj == 0),
                            stop=(kj == kmax - 1),
                        )
                        nc.tensor.matmul(
                            DN[h][:, off:],
                            ones_sb[:],
                            e[:, off:],
                            start=(kj == 0),
                            stop=(kj == kmax - 1),
                        )
                        if i + DEPTH < len(seq):
                            issue_st(i + DEPTH)
                        if pending and (a >= 0 or kj <= 1):
                            ss, oj = pending.pop(0)
                            outproj_tile(OTprev, qj - 1, ss, oj, pend_ysbs, "dve")
                        if kj == kmax - 1:
                            normalize(h)
                    for ss, oj in pending:
                        outproj_tile(OTprev, qj - 1, ss, oj, pend_ysbs, "act")
                    if mode == "causal":
                        if qj == SJ - 1:
                            outproj_block(OT, qj, tail=True)
                    else:
                        outproj_block(OT, qj, tail=(qj == SJ - 1))
                    OTprev = OT
    nc.compile()
    return nc


_PROGRAM_CACHE = {}


def _get_program(mode):
    if mode not in _PROGRAM_CACHE:
        _PROGRAM_CACHE[mode] = build_program(mode)
    return _PROGRAM_CACHE[mode]


def _detect_mode(attn_mask):
    m = np.asarray(attn_mask).reshape(S, S)
    if (m == np.tril(np.ones((S, S), m.dtype))).all():
        return "causal"
    if (m != 0).all():
        return "full"
    return "bias"


def _diag_bias():
    a = np.arange(4)[:, None, None]
    p = np.arange(128)[None, :, None]
    t = np.arange(512)[None, None, :]
    return np.where(128 * a + p <= t, 0.0, NEG).astype(np.float32)


def _bf16(a):
    return np.ascontiguousarray(a).astype(NPBF16)


def build_in_maps(inputs, mode):
    x = np.asarray(inputs["x"], np.float32)
    fcos = np.asarray(inputs["fcos"], np.float32)
    fsin = np.asarray(inputs["fsin"], np.float32)
    Wq, bq = np.asarray(inputs["Wq"], np.float32), np.asarray(inputs["bq"], np.float32)
    Wk, bk = np.asarray(inputs["Wk"], np.float32), np.asarray(inputs["bk"], np.float32)
    Wv, bv = np.asarray(inputs["Wv"], np.float32), np.asarray(inputs["bv"], np.float32)
    Wo = np.asarray(inputs["Wo"], np.float32)
    attn_mask = inputs["attn_mask"]

    sc = 1.0 / math.sqrt(HD)
    sgn = np.where(np.arange(HD) % 2 == 0, -1.0, 1.0).astype(np.float32)[:, None]
    shared = {
        "cosT": _bf16(fcos.T),
        "sinT": _bf16(fsin.T * sgn),
    }
    if mode == "causal":
        shared["dbias"] = _diag_bias()
    elif mode == "bias":
        m = np.asarray(attn_mask).reshape(S, S)
        shared["fbias"] = np.ascontiguousarray(
            np.where(m.T == 0, NEG, 0.0).astype(np.float32)
        )

    in_maps = []
    for c in range(NCORES):
        b, hg = divmod(c, HG)
        rows = slice(DLOC * hg, DLOC * (hg + 1))
        in_maps.append(
            {
                "xT": _bf16(x[b].T),
                "wqT": _bf16((Wq[rows] * sc).T),
                "wkT": _bf16(Wk[rows].T),
                "wvT": _bf16(Wv[rows].T),
                "woT": _bf16(Wo[:, rows].T),
                "bqT": np.ascontiguousarray((bq[rows] * sc).reshape(HPG, 128).T),
                "bkT": np.ascontiguousarray(bk[rows].reshape(HPG, 128).T),
                "bv": np.ascontiguousarray(
                    np.broadcast_to(bv[rows].reshape(1, DLOC), (128, DLOC))
                ).astype(np.float32),
                **shared,
            }
        )
    return in_maps


def postprocess_y(y):
    return np.asarray(y, np.float32)


def kernel(**inputs) -> np.ndarray:
    from concourse.bass_utils import run_bass_kernel_spmd

    mode = _detect_mode(inputs["attn_mask"])
    nc = _get_program(mode)
    in_maps = build_in_maps(inputs, mode)
    bo = np.asarray(inputs["bo"], np.float32)

    trace = bool(int(os.environ.get("KERNEL_TRACE", "0")))
    try:
        res = run_bass_kernel_spmd(nc, in_maps, list(range(NCORES)), trace=trace)
    except ModuleNotFoundError:
        # environments without the NTFF profile hook can still execute
        trace = False
        res = run_bass_kernel_spmd(nc, in_maps, list(range(NCORES)), trace=False)
    if trace and res.exec_time_ns is not None:
        print(f"HW exec time: {res.exec_time_ns} ns")
        globals()["LAST_EXEC_NS"] = res.exec_time_ns
        globals()["LAST_RESULTS"] = res

    out = np.zeros((B, S, H), np.float32)
    for c in range(NCORES):
        out[c // HG] += postprocess_y(res.results[c]["y"])
    out += bo
    return out
